# revision 14
# baseline (speedup 1.0000x reference)
"""GreenTF filterbank (strided sinusoid conv) on 8 trn2 NeuronCores.

reference:  k = kernel*envelope/SR;  frames = im2col(pad(wav), K=2048, stride=16)
            spec = einsum('btk,fk->bft', frames, k) * sqrt(8)/(sum(envelope)/SR)
            returns (spec[:, :8001], spec[:, 8001:])   # each [2, 8001, 2000]

Strategy: shard the 8001 frequency bands across the 8 cores (sin+cos rows of
the same band live together) and run the GEMM in fp8e4 with DoubleRow matmuls
(0.5 cyc/row, 2x fp16 PE throughput).  Accuracy: taps are weighted by the
decaying envelope, so the high-envelope (recent) taps get an fp8 hi+lo
split (3 DoubleRow terms per k-tile pair) while low-envelope taps run single
fp8; the split point h is chosen per input from an exact norm-based error
estimate.  Frames are never materialized for DMA: a Toeplitz SBUF layout
x[p, i, j] = pad[p + 16 j + 128 i] lets every matmul slice the same small
resident tensor, so per-core input DMA is ~7 MB instead of ~25 MB.

Falls back to the previous f16 GEMM variant when the estimate says fp8
cannot hit the accuracy budget for the given inputs.
"""

import math
import os

os.environ.setdefault("MYCRO_LOCAL_CACHE", "1")

import ml_dtypes
import numpy as np

import concourse.bass as bass  # noqa: F401  (engine handles live on the Bacc object)
import concourse.mybir as mybir
from concourse import bacc
from concourse.bass_utils import run_bass_kernel_spmd
from concourse.tile import TileContext

SR = 16000
KSIZE = 2048
F = 8001          # frequencies; rows 0:F sin, F:2F cos
B = 2
T_OUT = 2000
STRIDE = 16
N_CORES = 8
NT = B * T_OUT    # 4000 columns, batch-major
CHUNK = 500       # t-columns per matmul (PSUM free dim), 8 chunks
N_CHUNKS = NT // CHUNK
KT_TILES = KSIZE // 128   # 16 contraction tiles
FT_TILES = 16             # 2048 padded f-rows per core
FPAD = FT_TILES * 128
N_PAIRS = KT_TILES // 2   # 8 DoubleRow k-tile pairs
JB = 2136                 # Toeplitz j-extent per batch (j = t + 16*pair)
JTOT = B * JB
PADL = 34432              # 2047 left pad + 32000 + slack for the i=1 plane

# f-band sizes per core: 8001 = 1001 + 7*1000
_BAND = [1001] + [1000] * 7
_OFF = np.cumsum([0] + _BAND)

E4 = ml_dtypes.float8_e4m3   # == mybir.dt.np(float8e4); max normal 240

VARIANT = os.environ.get("GREENTF_VARIANT", "fp8")  # "fp8" | "f16" | "f32r"
H_FORCE = os.environ.get("GREENTF_H")                # force hi/lo pair count
ERR_BUDGET = 1.45e-2                                 # harness gate is 2e-2
TRACE = bool(int(os.environ.get("GREENTF_TRACE", "0")))

_prog_cache = {}


# --------------------------------------------------------------------------
# fp8 DoubleRow variant
# --------------------------------------------------------------------------

def _build_program_fp8(nf16, post):
    """Mixed GEMM: (16-nf16) old k-tiles as fp8 DoubleRow pairs, nf16 recent
    k-tiles in f16.  On TRN2 the PE streams ~1 col/cycle for every dtype, so
    cost == instruction count: 8 + nf16/2 per (ft, chunk)."""
    nc = bacc.Bacc()
    fp32 = mybir.dt.float32
    f8 = mybir.dt.float8e4
    f16 = mybir.dt.float16
    DR = mybir.MatmulPerfMode.DoubleRow
    npair = (KT_TILES - nf16) // 2          # fp8 DR pairs (k-tiles 0..)
    kt16 = list(range(KT_TILES - nf16, KT_TILES))

    xh_d = nc.dram_tensor("xh", [128, 2 * JTOT], f8, kind="ExternalInput").ap()
    x16_d = nc.dram_tensor("x16", [128, JTOT], f16, kind="ExternalInput").ap()
    wh_d = nc.dram_tensor("wh", [max(npair, 1) * 128, 2 * FPAD], f8,
                          kind="ExternalInput").ap()
    w16_d = nc.dram_tensor("w16", [max(nf16, 1) * 128, FPAD], f16,
                           kind="ExternalInput").ap()
    out_d = nc.dram_tensor("out", [FPAD, NT], f16, kind="ExternalOutput").ap()

    with TileContext(nc) as tc:
        with (
            tc.tile_pool(name="xpl", bufs=1) as x_p,
            tc.tile_pool(name="wts", bufs=1) as w_p,
            tc.tile_pool(name="ostage", bufs=4) as ostage_p,
            tc.tile_pool(name="ps", bufs=4, space="PSUM") as ps_p,
        ):
            # x16 lands first: the f16 ops run first in each ft, so the fp8
            # Toeplitz planes can arrive while ft 0's f16 groups stream
            xh_t = x_p.tile([128, 2, JTOT], f8, tag="xh")
            x16_t = x_p.tile([128, JTOT], f16, tag="x16")
            nc.sync.dma_start(out=x16_t[:], in_=x16_d)

            # PE warmup during the initial DMA wait: junk matmuls on a
            # memset scratch tile ramp the PE p-state toward 2.4 GHz
            warm = x_p.tile([128, 512], f8, tag="warm")
            nc.vector.memset(warm[:], 0.0)
            wps = ps_p.tile([128, 2, 512], fp32, name="wps", tag="ps")
            for w in range(24):
                nc.tensor.matmul(wps[:, 0, :], warm[:, 0:128], warm[:],
                                 start=(w == 0), stop=(w == 23))

            wh_t, w16_t = {}, {}
            order = [("f16", kt) for kt in kt16]
            order += [("dr", i) for i in range(npair)]
            for kind, idx in order:
                if kind == "f16":
                    t = w_p.tile([128, FPAD], f16, tag=f"w16_{idx}")
                    r = idx - (KT_TILES - nf16)
                    nc.sync.dma_start(out=t[:],
                                      in_=w16_d[r * 128:(r + 1) * 128, :])
                    w16_t[idx] = t
                elif idx == 0:
                    # fp8 planes + first DR pair queue after the f16 set
                    nc.sync.dma_start(out=xh_t[:], in_=xh_d)
            for kind, idx in order:
                if kind == "dr":
                    t = w_p.tile([128, 2, FPAD], f8, tag=f"wh{idx}")
                    nc.sync.dma_start(out=t[:],
                                      in_=wh_d[idx * 128:(idx + 1) * 128, :])
                    wh_t[idx] = t
            ops = order
            n_ops = len(ops)

            def mm(pt_ap, ft, c, oi):
                kind, idx = ops[oi]
                b, ci = divmod(c, N_CHUNKS // B)
                fsl = slice(ft * 128, (ft + 1) * 128)
                start, stop = (oi == 0), (oi == n_ops - 1)
                if kind == "dr":
                    j0 = JB * b + CHUNK * ci + 16 * idx
                    nc.tensor.matmul(pt_ap, wh_t[idx][:, :, fsl],
                                     xh_t[:, :, j0:j0 + CHUNK],
                                     start=start, stop=stop, perf_mode=DR)
                else:
                    j0 = JB * b + CHUNK * ci + 8 * idx
                    nc.tensor.matmul(pt_ap, w16_t[idx][:, fsl],
                                     x16_t[:, j0:j0 + CHUNK],
                                     start=start, stop=stop)

            # PSUM as four 2-bank sets: each set's two accumulations finish
            # and evict together, so banks free in simultaneous waves and the
            # scheduler keeps the op-major order (8 same-weight matmuls per
            # LDWEIGHTS) instead of racing depth-first down per-bank chains.
            def ev_copy(s, dst, src):
                # GPSIMD cannot read PSUM; alternate Activation / DVE
                if s % 2 == 0:
                    nc.scalar.mul(dst, src, post)
                else:
                    nc.vector.tensor_scalar_mul(dst, src, post)

            for ft in range(FT_TILES):
                sets = [ps_p.tile([128, 2, 512], fp32, tag="ps",
                                  name=f"ps{ft}_{s}") for s in range(4)]
                for oi in range(n_ops):
                    for c in range(N_CHUNKS):
                        mm(sets[c // 2][:, c % 2, 0:CHUNK], ft, c, oi)
                last = ft == FT_TILES - 1
                for s in range(4):
                    ot = ostage_p.tile([128, 2 * CHUNK], f16, tag="ostage")
                    if last:
                        # drain the final tile per bank so copies and DMAs
                        # overlap the remaining matmuls instead of queuing
                        # after the last one
                        for half in range(2):
                            ev_copy(s + half, ot[:, half * CHUNK:(half + 1) * CHUNK],
                                    sets[s][:, half, 0:CHUNK])
                            nc.sync.dma_start(
                                out=out_d[ft * 128:(ft + 1) * 128,
                                          (2 * s + half) * CHUNK:
                                          (2 * s + half + 1) * CHUNK],
                                in_=ot[:, half * CHUNK:(half + 1) * CHUNK],
                            )
                    else:
                        ev_copy(s, ot[:], sets[s][:, :, 0:CHUNK])
                        nc.sync.dma_start(
                            out=out_d[ft * 128:(ft + 1) * 128,
                                      s * 2 * CHUNK:(s + 1) * 2 * CHUNK],
                            in_=ot[:],
                        )

    nc.finalize()
    return nc


def _pack_pairs(ktap):
    """[K taps, FPAD f] fp8 -> [N_PAIRS*128, 2*FPAD] DoubleRow pair layout."""
    a = np.ascontiguousarray(ktap).reshape(N_PAIRS, 2, 128, FPAD)
    return np.ascontiguousarray(a.transpose(0, 2, 1, 3)).reshape(
        N_PAIRS * 128, 2 * FPAD)


def _planes(src):
    """[B, PADL] fp8 -> [128, 2, JTOT] Toeplitz planes x[p,i,j]=src[b,p+16j+128i]."""
    out = np.zeros((128, 2, JTOT), E4)
    for b in range(B):
        for i in range(2):
            av = np.lib.stride_tricks.as_strided(
                src[b, 128 * i:], shape=(128, JB), strides=(1, 16))
            out[:, i, JB * b:JB * (b + 1)] = av
    return out


def _plane1(src):
    """[B, PADL] f16 -> [128, JTOT] Toeplitz x[p,j]=src[b,p+16j]."""
    out = np.zeros((128, JTOT), np.float16)
    st = src.strides[1]
    for b in range(B):
        av = np.lib.stride_tricks.as_strided(
            src[b], shape=(128, JB), strides=(st, 16 * st))
        out[:, JB * b:JB * (b + 1)] = av
    return out


def _kernel_fp8(wav, kernel, envelope):
    env64 = envelope.astype(np.float64)
    scale = math.sqrt(8.0) / (env64.sum() / SR)
    kw = (kernel.astype(np.float64) * (env64 / SR * scale)[None, :]).astype(
        np.float32)                                     # [2F, K] exact weights

    akmax = float(np.abs(kw).max())
    awmax = float(np.abs(wav).max())
    CK = 2.0 ** math.floor(math.log2(224.0 / akmax)) if akmax > 0 else 1.0
    CX = 2.0 ** math.floor(math.log2(224.0 / awmax)) if awmax > 0 else 1.0

    kq = kw * np.float32(CK)
    khi8 = kq.astype(E4)
    k16 = kq.astype(np.float16)

    padq = np.zeros((B, PADL), np.float32)
    padq[:, KSIZE - 1:KSIZE - 1 + wav.shape[1]] = wav * np.float32(CX)
    xhi8 = padq.astype(E4)
    x16p = padq.astype(np.float16)

    # ---- norm-based exact error estimate per tap -> choose nf16 ----
    s = padq.strides
    sw = np.lib.stride_tricks.as_strided(
        padq, shape=(B, T_OUT, KSIZE), strides=(s[0], 16 * s[1], s[1]))
    Sx = np.einsum('bti,bti->i', sw, sw, optimize=True).astype(np.float64)

    def wsum(arr):
        st = arr.strides
        v = np.lib.stride_tricks.as_strided(
            arr, shape=(B, T_OUT, KSIZE), strides=(st[0], 16 * st[1], st[1]))
        return np.einsum('bti,bti->i', v, v, optimize=True).astype(np.float64)

    d1 = padq - xhi8.astype(np.float32)
    SD1 = wsum(d1)
    d2 = padq - x16p.astype(np.float32)
    SD2 = wsum(d2)
    Bk = np.einsum('fi,fi->i', kq, kq, optimize=True).astype(np.float64)
    e1 = kq - khi8.astype(np.float32)
    A1 = np.einsum('fi,fi->i', e1, e1, optimize=True).astype(np.float64)
    e2 = kq - k16.astype(np.float32)
    A2 = np.einsum('fi,fi->i', e2, e2, optimize=True).astype(np.float64)
    den2 = float(np.sum(Bk * Sx))

    def est(nf16):
        nR = 128 * nf16
        S, R = slice(0, KSIZE - nR), slice(KSIZE - nR, KSIZE)
        num2 = float(np.sum(A1[S] * Sx[S] + Bk[S] * SD1[S]))
        num2 += float(np.sum(A2[R] * Sx[R] + Bk[R] * SD2[R]))
        return math.sqrt(num2 / den2) if den2 > 0 else 0.0

    nf16 = None
    if H_FORCE is not None:
        nf16 = int(H_FORCE)
    else:
        for cand in (4, 6, 8, 10, 12, 14, 16):
            if est(cand) <= ERR_BUDGET:
                nf16 = cand
                break
    if nf16 is None:
        return None  # caller falls back to f16 variant
    if TRACE:
        print(f"fp8 variant: nf16={nf16} est_rel_err={est(nf16):.3e} "
              f"CK=2^{int(math.log2(CK))} CX=2^{int(math.log2(CX))}")

    xh_host = _planes(xhi8).reshape(128, 2 * JTOT)
    x16_host = _plane1(x16p)
    npair = (KT_TILES - nf16) // 2

    in_maps = []
    for c in range(N_CORES):
        f0, fc = int(_OFF[c]), _BAND[c]
        ktap_hi = np.zeros((KSIZE, FPAD), E4)
        ktap_hi[:, :fc] = khi8[f0:f0 + fc].T
        ktap_hi[:, fc:2 * fc] = khi8[F + f0:F + f0 + fc].T
        wh = _pack_pairs(ktap_hi)[:max(npair, 1) * 128]
        ktap_16 = np.zeros((KSIZE, FPAD), np.float16)
        ktap_16[:, :fc] = k16[f0:f0 + fc].T
        ktap_16[:, fc:2 * fc] = k16[F + f0:F + f0 + fc].T
        w16 = np.ascontiguousarray(ktap_16[(KT_TILES - nf16) * 128:])
        if nf16 == 0:
            w16 = np.zeros((128, FPAD), np.float16)
        in_maps.append({"xh": xh_host, "x16": x16_host, "wh": wh, "w16": w16})

    post = 1.0 / (CK * CX)
    key = ("fp8", nf16, CK, CX)
    if key not in _prog_cache:
        _prog_cache[key] = _build_program_fp8(nf16, post)
    nc = _prog_cache[key]

    kwargs = {}
    if TRACE:
        kwargs["tmpdir"] = os.environ.get("GREENTF_TRACE_DIR") or None
    res = run_bass_kernel_spmd(nc, in_maps, list(range(N_CORES)), trace=TRACE,
                               **kwargs)
    if TRACE:
        print(f"HW exec time: {res.exec_time_ns} ns "
              f"(mean {res.mean_exec_time_ns} ns, core {res.max_exec_time_core_id})")

    sspec = np.empty((B, F, T_OUT), np.float32)
    cspec = np.empty((B, F, T_OUT), np.float32)
    for c in range(N_CORES):
        f0, fc = int(_OFF[c]), _BAND[c]
        o = res.results[c]["out"]
        for b in range(B):
            cols = slice(b * T_OUT, (b + 1) * T_OUT)
            sspec[b, f0:f0 + fc] = o[:fc, cols]
            cspec[b, f0:f0 + fc] = o[fc:2 * fc, cols]
    return sspec, cspec


# --------------------------------------------------------------------------
# f16 / f32r fallback variant (previous baseline, known-good)
# --------------------------------------------------------------------------

def _build_program_f16(variant):
    nc = bacc.Bacc()
    fp32 = mybir.dt.float32
    f16 = variant == "f16"
    cdt = mybir.dt.float16 if f16 else mybir.dt.float32r

    kt_d = nc.dram_tensor("kt", [KSIZE, FPAD],
                          mybir.dt.float16 if f16 else fp32,
                          kind="ExternalInput").ap()
    fr_d = nc.dram_tensor("fr", [KSIZE, NT], cdt, kind="ExternalInput").ap()
    env_d = nc.dram_tensor("env", [128, KT_TILES], fp32, kind="ExternalInput").ap()
    out_d = nc.dram_tensor("out", [FPAD, NT], fp32, kind="ExternalOutput").ap()

    post = (1.0 / 256.0) if f16 else 1.0

    with TileContext(nc) as tc:
        with (
            tc.tile_pool(name="ktres", bufs=1) as ktres_p,
            tc.tile_pool(name="env", bufs=1) as env_p,
            tc.tile_pool(name="fr", bufs=2) as fr_p,
            tc.tile_pool(name="ostage", bufs=4) as ostage_p,
            tc.tile_pool(name="ps", bufs=8, space="PSUM") as ps_p,
        ):
            env_t = env_p.tile([128, KT_TILES], fp32)
            nc.sync.dma_start(out=env_t[:], in_=env_d)

            warm = env_p.tile([128, 512], cdt, name="warm")
            nc.vector.memset(warm[:], 0.0)
            wps = ps_p.tile([128, 512], fp32, name="wps", tag="ps")
            for w in range(24):
                nc.tensor.matmul(wps[:], warm[:, 0:128], warm[:],
                                 start=(w == 0), stop=(w == 23))

            frt0 = fr_p.tile([128, KT_TILES, CHUNK], cdt, tag="fr")
            ktres = []
            for kt in range(KT_TILES):
                t = ktres_p.tile([128, FPAD], cdt, tag=f"kt{kt}")
                if f16:
                    nc.sync.dma_start(out=t[:], in_=kt_d[kt * 128:(kt + 1) * 128, :])
                    src = t[:]
                else:
                    nc.sync.dma_start(
                        out=t[:].bitcast(fp32), in_=kt_d[kt * 128:(kt + 1) * 128, :]
                    )
                    src = t[:].bitcast(fp32)
                nc.sync.dma_start(
                    out=frt0[:, kt, :], in_=fr_d[kt * 128:(kt + 1) * 128, 0:CHUNK]
                )
                nc.vector.tensor_scalar_mul(t[:], src, env_t[:, kt:kt + 1])
                ktres.append(t)

            def mm(pt, ft, kt, frt):
                nc.tensor.matmul(
                    pt[:],
                    ktres[kt][:, ft * 128:(ft + 1) * 128],
                    frt[:, kt, :],
                    start=(kt == 0),
                    stop=(kt == KT_TILES - 1),
                )

            def evict(pt, ft, c):
                ot = ostage_p.tile([128, CHUNK], fp32, tag="ostage")
                nc.any.tensor_scalar_mul(ot[:], pt[:], post)
                nc.sync.dma_start(
                    out=out_d[ft * 128:(ft + 1) * 128,
                              c * CHUNK:(c + 1) * CHUNK],
                    in_=ot[:],
                )

            for g in range(2):
                fts = list(range(g * 8, g * 8 + 8))
                pts = {}
                for ft in fts:
                    pts[ft] = ps_p.tile([128, CHUNK], fp32, tag="ps", name=f"pt{ft}")
                for kt in range(KT_TILES):
                    for ft in fts:
                        mm(pts[ft], ft, kt, frt0)
                for ft in fts:
                    evict(pts[ft], ft, 0)

            for c in range(1, N_CHUNKS):
                frt = fr_p.tile([128, KT_TILES, CHUNK], cdt, tag="fr")
                cs = slice(c * CHUNK, (c + 1) * CHUNK)
                for kt in range(KT_TILES):
                    nc.sync.dma_start(
                        out=frt[:, kt, :], in_=fr_d[kt * 128:(kt + 1) * 128, cs]
                    )
                for ft in range(FT_TILES):
                    pt = ps_p.tile([128, CHUNK], fp32, tag="ps")
                    for kt in range(KT_TILES):
                        mm(pt, ft, kt, frt)
                    evict(pt, ft, c)

    nc.finalize()
    return nc


def _round_f32r(x: np.ndarray) -> np.ndarray:
    u = x.astype(np.float32).view(np.uint32).copy()
    lsb = (u >> 12) & 1
    u += 0x7FF + lsb
    u &= 0xFFFFF000
    return u.view(np.float32)


def _kernel_f16(wav, kernel, envelope, variant):
    env = envelope.astype(np.float64)
    scale = math.sqrt(8.0) / (env.sum() / SR)
    env_dev = (env / SR * scale * (256.0 if variant == "f16" else 1.0)).astype(
        np.float32
    )
    env2d = np.ascontiguousarray(env_dev.reshape(KT_TILES, 128).T)  # [128, kt]

    pad = np.zeros((B, KSIZE - 1 + wav.shape[1]), np.float32)
    pad[:, KSIZE - 1:] = wav
    frames = np.lib.stride_tricks.sliding_window_view(pad, KSIZE, axis=1)[:, ::STRIDE]
    fr = np.ascontiguousarray(frames.transpose(2, 0, 1).reshape(KSIZE, NT))
    fr = fr.astype(np.float16) if variant == "f16" else _round_f32r(fr)

    in_maps = []
    for c in range(N_CORES):
        f0, fc = int(_OFF[c]), _BAND[c]
        kt = np.zeros((KSIZE, FPAD), np.float32)
        kt[:, :fc] = kernel[f0:f0 + fc].T
        kt[:, fc:2 * fc] = kernel[F + f0:F + f0 + fc].T
        if variant == "f16":
            kt = kt.astype(np.float16)
        in_maps.append({"kt": kt, "fr": fr, "env": env2d})

    key = variant
    if key not in _prog_cache:
        _prog_cache[key] = _build_program_f16(variant)
    nc = _prog_cache[key]

    kwargs = {}
    if TRACE:
        kwargs["tmpdir"] = os.environ.get("GREENTF_TRACE_DIR") or None
    res = run_bass_kernel_spmd(nc, in_maps, list(range(N_CORES)), trace=TRACE, **kwargs)
    if TRACE:
        print(f"HW exec time: {res.exec_time_ns} ns "
              f"(mean {res.mean_exec_time_ns} ns, core {res.max_exec_time_core_id})")

    sspec = np.empty((B, F, T_OUT), np.float32)
    cspec = np.empty((B, F, T_OUT), np.float32)
    for c in range(N_CORES):
        f0, fc = int(_OFF[c]), _BAND[c]
        o = res.results[c]["out"]
        for b in range(B):
            cols = slice(b * T_OUT, (b + 1) * T_OUT)
            sspec[b, f0:f0 + fc] = o[:fc, cols]
            cspec[b, f0:f0 + fc] = o[fc:2 * fc, cols]
    return sspec, cspec


def kernel(wav: np.ndarray, kernel: np.ndarray, envelope: np.ndarray):
    assert wav.shape == (B, T_OUT * STRIDE) and kernel.shape == (2 * F, KSIZE)
    if VARIANT == "fp8":
        out = _kernel_fp8(wav, kernel, envelope)
        if out is not None:
            return out
        return _kernel_f16(wav, kernel, envelope, "f16")
    return _kernel_f16(wav, kernel, envelope, VARIANT)
